# revision 21
# baseline (speedup 1.0000x reference)
"""Causal multi-head attention on 8 TRN2 NeuronCores.

Sharding: core c -> (batch b = c // 4, head-group g = c % 4, heads 4g..4g+3).
Each core computes its 4 heads' attention plus the partial output projection
(out_partial^T = W_O_g^T @ z_g^T, [1024, 2048] fp32). Host sums the 4 head-group
partials per batch, adds b_O, transposes back.

v9 schedule (v8 was 168.7us):
  - Dual-ring input DMA: xT quarters on the Sync HWDGE ring, weights on the
    Activation HWDGE ring, with the first wk/wq/xT-q0 transfers split in
    halves so the ht=0 qi=0 q/k chains start ~5us earlier.
  - PE p-state warmup: ~8 dummy matmuls on zero tiles during the DMA window
    so the first real matmul runs at full 2.4GHz (idle PE drops to 1.2GHz and
    needs ~3us of continuous work to ramp back).
  - Causal mask folded into the scores PSUM accumulation as an additive
    -1e5 matmul (stationary=I, moving=Mneg, start=False) on each diagonal
    k-tile: exp feeds AV directly, GpSimd is out of the critical path.
  - Out-projection PSUM->SBUF casts on DVE instead of ACT (ACT was the
    co-bottleneck at 135us busy with exp=75us + identity=21us).
  - Denominator broadcast as ONE matmul: v_aug has ones-columns 64 (hb0) and
    65 (hb1), so den0/den1 land on PSUM rows 64/65 of the two zpair banks;
    two lane-aligned copies then a single [2,128]-stationary matmul
    broadcasts both into one [128, 512] PSUM tile; one reciprocal, 2 mults.
  - PSUM: sc [128,2,512]x2 | zpair [128,2,512]x1 | ms [128,512]x2 = 8 banks.
    Hard-won rule: matmul start=True clears the whole bank row of every
    partition it writes, so interleaved accumulation chains may share a bank
    only at disjoint PARTITION ranges, never disjoint column ranges.
  - Per step: scores FIRST (only waits on the spool slot, i.e. ACT pace),
    then fillers - a filler emitted before scores couples the scores stream
    to the filler's exp deps through the in-order PE queue.
"""

import os
import sys

import numpy as np

for _p in ("/opt/trn_rl_repo", "/opt/pypackages"):
    if _p not in sys.path and os.path.isdir(_p):
        sys.path.append(_p)

import ml_dtypes  # noqa: E402

import concourse.bass as bass  # noqa: E402
import concourse.mybir as mybir  # noqa: E402
import concourse.tile as tile  # noqa: E402
from concourse import bacc  # noqa: E402
from concourse.bass_utils import run_bass_kernel_spmd  # noqa: E402

BF16 = mybir.dt.bfloat16
F32 = mybir.dt.float32
F32R = mybir.dt.float32r
NPBF16 = ml_dtypes.bfloat16

B = 2
S = 2048
D = 1024
N_HEADS = 16
DH = 64
NH_CORE = 4          # heads per core
HE = NH_CORE * DH    # 256 concatenated head dims per core
QB = 512             # q block (moving operand width)
NQ = S // QB         # 4
NKT = S // 128       # 16 k-position tiles
NDT = D // 128       # 8 d_model tiles

LAST_RESULT = None
_GRAPH_CACHE = {}


def _emit(nc, tc, ctx, bias_qkv):
    xT = nc.declare_dram_parameter("xT", [D, S], BF16, isOutput=False).ap()
    wq = nc.declare_dram_parameter("wq", [D, HE], BF16, isOutput=False).ap()
    wk = nc.declare_dram_parameter("wk", [D, HE], BF16, isOutput=False).ap()
    wv = nc.declare_dram_parameter("wv", [D, HE], BF16, isOutput=False).ap()
    wo = nc.declare_dram_parameter("wo", [HE, D], BF16, isOutput=False).ap()
    masks = nc.declare_dram_parameter("masks", [128, 2 * 128], BF16, isOutput=False).ap()
    if bias_qkv:
        bq = nc.declare_dram_parameter("bq", [HE], F32, isOutput=False).ap()
        bk = nc.declare_dram_parameter("bk", [HE], F32, isOutput=False).ap()
        bv = nc.declare_dram_parameter("bv", [HE], F32, isOutput=False).ap()
    out = nc.declare_dram_parameter("out", [D, S], BF16, isOutput=True).ap()

    consts = ctx.enter_context(tc.tile_pool(name="consts", bufs=1))

    xT_sb = consts.tile([128, NDT, S], BF16)
    wq_sb = consts.tile([128, NDT, HE], BF16)
    wk_sb = consts.tile([128, NDT, HE], BF16)
    wv_sb = consts.tile([128, NDT, HE], BF16)
    wo_sb = consts.tile([128, 2, D], BF16)
    mask2_sb = consts.tile([128, 2, 128], BF16)

    # Activation-ring DMAs: consts + weights, first-needed split in halves.
    nc.scalar.dma_start(out=mask2_sb, in_=masks)
    nc.scalar.dma_start(out=wk_sb[:, 0:4, :],
                        in_=wk[0:512, :].rearrange("(t p) e -> p t e", p=128))
    nc.scalar.dma_start(out=wk_sb[:, 4:8, :],
                        in_=wk[512:1024, :].rearrange("(t p) e -> p t e", p=128))
    nc.scalar.dma_start(out=wq_sb[:, 0:4, :],
                        in_=wq[0:512, :].rearrange("(t p) e -> p t e", p=128))
    nc.scalar.dma_start(out=wq_sb[:, 4:8, :],
                        in_=wq[512:1024, :].rearrange("(t p) e -> p t e", p=128))

    # Sync-ring DMAs: xT in sequence-column quarters (q0 in d-halves).
    nc.sync.dma_start(out=xT_sb[:, 0:4, 0:QB],
                      in_=xT[0:512, 0:QB].rearrange("(t p) s -> p t s", p=128))
    nc.sync.dma_start(out=xT_sb[:, 4:8, 0:QB],
                      in_=xT[512:1024, 0:QB].rearrange("(t p) s -> p t s", p=128))
    for qs in range(1, 4):
        nc.sync.dma_start(
            out=xT_sb[:, :, qs * QB:(qs + 1) * QB],
            in_=xT[:, qs * QB:(qs + 1) * QB].rearrange("(t p) s -> p t s", p=128))

    # ACT exp table pre-load (~2.7us) during the input-DMA window, after the
    # first weight-DMA issues so it doesn't delay them on the ACT queue.
    warm_i = consts.tile([128, 8], F32)
    nc.vector.memset(warm_i, 0.0)
    warm_o = consts.tile([128, 8], BF16)
    nc.scalar.activation(out=warm_o, in_=warm_i,
                         func=mybir.ActivationFunctionType.Exp, scale=0.125)

    nc.scalar.dma_start(out=wv_sb, in_=wv.rearrange("(t p) e -> p t e", p=128))
    nc.scalar.dma_start(out=wo_sb, in_=wo.rearrange("(t p) d -> p t d", p=128))

    if bias_qkv:
        bq_sb = consts.tile([128, 2], F32)
        nc.scalar.dma_start(out=bq_sb, in_=bq.rearrange("(t p) -> p t", p=128))
        bk_sb = consts.tile([128, 2], F32)
        nc.scalar.dma_start(out=bk_sb, in_=bk.rearrange("(t p) -> p t", p=128))
        bv_r = bv.rearrange("(a b e) -> a b e", a=2, b=2)
        bv_bcast = bass.AP(tensor=bv_r.tensor, offset=bv_r.offset,
                           ap=[[0, 128]] + list(bv_r.ap))
        bv_sb = consts.tile([128, 2, 2, DH], F32)
        nc.scalar.dma_start(out=bv_sb, in_=bv_bcast)

    qT_sb = consts.tile([128, 2, S], BF16)
    kT_sb = consts.tile([128, 2, S], BF16)
    zT_sb = consts.tile([128, 2, S], BF16)
    # v_aug [k, ht, hb, 65]: cols 0-63 = v, col 64 = 1 -> each head's softmax
    # denominator accumulates on PSUM row 64 of its own zpair bank.
    VA = DH + 1
    v_sb = consts.tile([128, NKT, 2, 2, VA], BF16)
    nc.vector.memset(v_sb[:, :, :, :, DH:VA], 1.0)

    # ones row (partition 64) for the denominator broadcast matmuls
    ones64 = consts.tile([128, DH], BF16)
    nc.vector.memset(ones64[64:65, :], 1.0)

    # PE p-state warmup fodder (zeros; overwritten semantics-free).
    warm_w = consts.tile([128, 128], BF16)
    nc.vector.memset(warm_w, 0.0)
    warm_m = consts.tile([128, QB], BF16)
    nc.vector.memset(warm_m, 0.0)

    # PSUM: sc 2x[128,2,512] (4 banks) + zpair 1x[128,2,512] (2) +
    # misc 2x[128,512] (2) = 8 banks.
    spool = ctx.enter_context(tc.tile_pool(name="spool", bufs=2, space="PSUM"))
    zpool = ctx.enter_context(tc.tile_pool(name="zpool", bufs=1, space="PSUM"))
    mpool = ctx.enter_context(tc.tile_pool(name="mpool", bufs=2, space="PSUM"))
    ppool = ctx.enter_context(tc.tile_pool(name="ppool", bufs=32))
    rpool = ctx.enter_context(tc.tile_pool(name="rpool", bufs=2))
    opool = ctx.enter_context(tc.tile_pool(name="opool", bufs=6))
    hpool = ctx.enter_context(tc.tile_pool(name="hpool", bufs=16))

    # ---------------- q/k projections ----------------
    def emit_qk_phase0():
        """ht=0 q/k chains emitted per sequence quarter, in the quarter's xT
        arrival order; each chain's cast follows immediately so the first
        scores/exp of the attention loop start while xT is still landing.
        Chain homes: qi0/qi1 -> sc pool, qi2 -> zpair, qi3 -> 2 ms tiles, so
        the scores(0,0) spool request only waits on qi0's cast."""
        hsl = slice(0, 128)

        def cast(dst, src_ap, bias_sb, qi):
            if bias_qkv:
                nc.scalar.activation(
                    out=dst[:, 0, qi * QB:(qi + 1) * QB], in_=src_ap,
                    func=mybir.ActivationFunctionType.Identity,
                    bias=bias_sb[:, 0:1])
            else:
                nc.vector.tensor_copy(
                    out=dst[:, 0, qi * QB:(qi + 1) * QB], in_=src_ap)

        for qi in range(NQ):
            if qi < 2:
                ch = spool.tile([128, 2, QB], F32, tag="sc", name=f"qk0c{qi}")
                kch, qch = ch[:, 0, :], ch[:, 1, :]
            elif qi == 2:
                ch = zpool.tile([128, 2, QB], F32, tag="zp", name="qk0c2")
                kch, qch = ch[:, 0, :], ch[:, 1, :]
            else:
                kch = mpool.tile([128, QB], F32, tag="ms", name="qk0c3k")
                qch = mpool.tile([128, QB], F32, tag="ms", name="qk0c3q")
            if qi == 0:
                # p-state warmup A: dummy matmuls with no DMA deps fill the
                # queue until the first wk/xT slices land.
                for wi in range(6):
                    nc.tensor.matmul(kch, warm_w, warm_m,
                                     start=(wi == 0), stop=(wi == 5))
            qsl = slice(qi * QB, (qi + 1) * QB)
            for t in range(4):
                nc.tensor.matmul(kch, wk_sb[:, t, hsl], xT_sb[:, t, qsl],
                                 start=(t == 0), stop=False)
            if qi == 0:
                # warmup B in the qch bank: fills the DMA wait between the
                # first xT/wk slices and the rest.
                for wi in range(3):
                    nc.tensor.matmul(qch, warm_w, warm_m,
                                     start=(wi == 0), stop=(wi == 2))
            for t in range(4, NDT):
                nc.tensor.matmul(kch, wk_sb[:, t, hsl], xT_sb[:, t, qsl],
                                 start=False, stop=(t == NDT - 1))
            cast(kT_sb, kch, bk_sb if bias_qkv else None, qi)
            for t in range(NDT):
                nc.tensor.matmul(qch, wq_sb[:, t, hsl], xT_sb[:, t, qsl],
                                 start=(t == 0), stop=(t == NDT - 1))
            cast(qT_sb, qch, bq_sb if bias_qkv else None, qi)

    def qk1_chunk_items():
        """ht=1 q/k chains as 8 short-lived contiguous chunks (one ms-pool
        tile each: 8 accumulating matmuls + an immediate cast). Short tile
        lifetime keeps the ms-pool cycling deadlock-free while these run as
        PE filler inside the attention loop (xT is fully resident by then)."""
        hsl = slice(128, 256)
        items = []
        for typ in ("k", "q"):
            for qi in range(NQ):
                def mk(typ=typ, qi=qi):
                    def f():
                        wsb = wk_sb if typ == "k" else wq_sb
                        dst = kT_sb if typ == "k" else qT_sb
                        ch = mpool.tile([128, QB], F32, tag="ms",
                                        name=f"qk1{typ}{qi}")
                        for t in range(NDT):
                            nc.tensor.matmul(
                                ch, wsb[:, t, hsl],
                                xT_sb[:, t, qi * QB:(qi + 1) * QB],
                                start=(t == 0), stop=(t == NDT - 1))
                        if bias_qkv:
                            bsb = bk_sb if typ == "k" else bq_sb
                            nc.scalar.activation(
                                out=dst[:, 1, qi * QB:(qi + 1) * QB], in_=ch,
                                func=mybir.ActivationFunctionType.Identity,
                                bias=bsb[:, 1:2])
                        else:
                            nc.vector.tensor_copy(
                                out=dst[:, 1, qi * QB:(qi + 1) * QB], in_=ch)
                    return f
                items.append(mk())
        return items

    # ---------------- V projection, per 2-k-tile group, t-outer ----------------
    # One PSUM bank per chain: matmul start=True clears the ENTIRE bank row of
    # every partition it writes, so two interleaved accumulation chains must
    # never share a bank in disjoint column ranges (disjoint partitions is ok).
    def v_group_items(g):
        tiles = {}
        items = []
        for t in range(NDT):
            def mk(t=t):
                def f():
                    if t == 0:
                        tiles[0] = mpool.tile([128, QB], F32, tag="ms",
                                              name=f"vg{g}a")
                        tiles[1] = mpool.tile([128, QB], F32, tag="ms",
                                              name=f"vg{g}b")
                    for i in range(2):
                        kt = 2 * g + i
                        nc.tensor.matmul(
                            tiles[i][:, 0:HE],
                            xT_sb[:, t, kt * 128:(kt + 1) * 128],
                            wv_sb[:, t, :],
                            start=(t == 0), stop=(t == NDT - 1))
                return f
            items.append(mk())

        def fcopy():
            for i in range(2):
                kt = 2 * g + i
                src = tiles[i][:, 0:HE].rearrange("p (a b e) -> p a b e",
                                                  a=2, b=2)
                if bias_qkv:
                    nc.vector.tensor_add(v_sb[:, kt, :, :, 0:DH], src, bv_sb)
                else:
                    nc.vector.tensor_copy(out=v_sb[:, kt, :, :, 0:DH], in_=src)
        items.append(fcopy)
        return items

    # ---------------- attention pieces ----------------
    def emit_scores_step(qi, ht, kj, pTs):
        qo = max(0, kj - 4 * qi) * 128
        ps = spool.tile([128, 2, QB], F32, tag="sc", name=f"sc{qi}{ht}{kj}")
        for hb in range(2):
            pb = hb * 64
            nc.tensor.matmul(
                ps[:, hb, qo:],
                kT_sb[pb:pb + 64, ht, kj * 128:(kj + 1) * 128],
                qT_sb[pb:pb + 64, ht, qi * QB + qo:(qi + 1) * QB],
                start=True, stop=True)
        pT = ppool.tile([128, 2, QB], BF16, tag="pT", name=f"pT{qi}{ht}{kj}")
        nc.scalar.activation(
            out=pT[:, :, qo:], in_=ps[:, :, qo:],
            func=mybir.ActivationFunctionType.Exp, scale=0.125)
        if kj >= 4 * qi:
            # causal mask only bites in the [128, 128] square at the diagonal
            # (GpSimd has slack; AV consumes pT a full iteration later, so
            # the extra latency hop is hidden)
            nc.gpsimd.tensor_mul(
                pT[:, :, qo:qo + 128], pT[:, :, qo:qo + 128], mask2_sb)
        pTs.append((pT, qo))

    def av_items(qi, ht, pTs):
        nk = 4 * qi + 4
        zref = {}
        items = []
        for kj in range(nk):
            def mk(kj=kj):
                def f():
                    if kj == 0:
                        zref["z"] = zpool.tile([128, 2, QB], F32, tag="zp",
                                               name=f"zp{qi}{ht}")
                    zp = zref["z"]
                    pT, qo = pTs[kj]
                    for hb in range(2):
                        nc.tensor.matmul(
                            zp[0:VA, hb, qo:],
                            v_sb[:, kj, ht, hb, 0:VA],
                            pT[:, hb, qo:],
                            start=(kj == 0), stop=(kj == nk - 1))
                return f
            items.append(mk())
        return items, zref

    def norm_items(qi, ht, zref):
        qsl = slice(qi * QB, (qi + 1) * QB)
        rref = {}

        # bf16 copy of z+den rows per head-pair half: the copies become zp's
        # ONLY readers, so the zpair PSUM banks free ~1.4us earlier and
        # AV(i+1) (zpool bufs=1) starts while the norm math proceeds on the
        # SBUF copy. Split per-hb for finer PE/DVE interleave. Everything is
        # lane-aligned at base 0 (SBUF x SBUF DVE ops require equal bases).
        def f_c0():
            zc = rpool.tile([128, 2, QB], BF16, tag="ds", name=f"zc{qi}{ht}")
            rref["d"] = zc
            nc.vector.tensor_copy(out=zc[0:DH + 1, 0, :],
                                  in_=zref["z"][0:DH + 1, 0, :])

        def f_ba():
            dpa = mpool.tile([128, QB], F32, tag="ms", name=f"dpa{qi}{ht}")
            rref["ba"] = dpa
            nc.tensor.matmul(dpa[0:DH, :], ones64[64:65, :],
                             rref["d"][64:65, 0, :], start=True, stop=True)

        def f_c1():
            nc.vector.tensor_copy(out=rref["d"][0:DH + 1, 1, :],
                                  in_=zref["z"][0:DH + 1, 1, :])

        def f_bb():
            dpb = mpool.tile([128, QB], F32, tag="ms", name=f"dpb{qi}{ht}")
            rref["bb"] = dpb
            nc.tensor.matmul(dpb[0:DH, :], ones64[64:65, :],
                             rref["d"][64:65, 1, :], start=True, stop=True)

        def f_m0():
            zc = rref["d"]
            bsb = rpool.tile([128, 2, QB], F32, tag="bs", name=f"bsb{qi}{ht}")
            rref["r"] = bsb
            nc.vector.reciprocal_approx_fast(bsb[0:DH, 0, :],
                                             rref["ba"][0:DH, :])
            nc.vector.tensor_mul(zT_sb[0:DH, ht, qsl], zc[0:DH, 0, :],
                                 bsb[0:DH, 0, :])

        def f_m1():
            zc = rref["d"]
            bsb = rref["r"]
            nc.vector.reciprocal_approx_fast(bsb[0:DH, 1, :],
                                             rref["bb"][0:DH, :])
            nc.vector.tensor_mul(zT_sb[DH:2 * DH, ht, qsl], zc[0:DH, 1, :],
                                 bsb[0:DH, 1, :])
        return [f_c0, f_ba, f_c1, f_bb, f_m0, f_m1]

    def proj_items(qi):
        qsl = slice(qi * QB, (qi + 1) * QB)
        items = []
        for dt in range(NDT):
            def mk(dt=dt):
                def f():
                    ops = mpool.tile([128, QB], F32, tag="ms",
                                     name=f"pj{qi}{dt}")
                    for t in range(2):
                        nc.tensor.matmul(
                            ops, wo_sb[:, t, dt * 128:(dt + 1) * 128],
                            zT_sb[:, t, qsl],
                            start=(t == 0), stop=(t == 1))
                    osb = opool.tile([128, QB], BF16, tag="ot",
                                     name=f"ot{qi}{dt}")
                    # alternate casts between DVE and ACT so neither queue
                    # backs up ahead of the norm chain
                    if dt % 2 == 0:
                        nc.vector.tensor_copy(out=osb, in_=ops)
                    else:
                        nc.scalar.activation(
                            out=osb, in_=ops,
                            func=mybir.ActivationFunctionType.Identity)
                    nc.sync.dma_start(out=out[dt * 128:(dt + 1) * 128, qsl],
                                      in_=osb)
                return f
            items.append(mk())
        return items

    # ht-split projection for the work-starved last iterations: the ht=0
    # half depends only on norm(qi,0) (done phases earlier), so it runs as
    # mid-kernel filler into SBUF bf16; the late half is then a single
    # matmul + fused add-cast per output tile.
    def proj_half0_items(qi, href):
        qsl = slice(qi * QB, (qi + 1) * QB)
        items = []
        for dt in range(NDT):
            def mk(dt=dt):
                def f():
                    hps = mpool.tile([128, QB], F32, tag="ms",
                                     name=f"ph{qi}{dt}")
                    nc.tensor.matmul(
                        hps, wo_sb[:, 0, dt * 128:(dt + 1) * 128],
                        zT_sb[:, 0, qsl], start=True, stop=True)
                    hsb = hpool.tile([128, QB], BF16, tag="hf",
                                     name=f"hf{qi}{dt}")
                    href[dt] = hsb
                    if dt % 2 == 0:
                        nc.vector.tensor_copy(out=hsb, in_=hps)
                    else:
                        nc.scalar.activation(
                            out=hsb, in_=hps,
                            func=mybir.ActivationFunctionType.Identity)
                return f
            items.append(mk())
        return items

    def proj_half1_items(qi, href):
        qsl = slice(qi * QB, (qi + 1) * QB)
        items = []
        for dt in range(NDT):
            def mk(dt=dt):
                def f():
                    ops = mpool.tile([128, QB], F32, tag="ms",
                                     name=f"pk{qi}{dt}")
                    nc.tensor.matmul(
                        ops, wo_sb[:, 1, dt * 128:(dt + 1) * 128],
                        zT_sb[:, 1, qsl], start=True, stop=True)
                    osb = opool.tile([128, QB], BF16, tag="ot",
                                     name=f"ot{qi}{dt}")
                    nc.vector.tensor_add(osb, ops, href[dt])
                    nc.sync.dma_start(out=out[dt * 128:(dt + 1) * 128, qsl],
                                      in_=osb)
                return f
            items.append(mk())
        return items

    def emit_iteration(qi, ht, fillers):
        # scores FIRST in each step: scores(s) only waits on the spool slot
        # (exp(s-2), i.e. ACT pace), keeping ACT fed; the fillers behind it
        # (AV(i-1)/projections/V) are dependency-clean by then. Emitting a
        # filler before scores couples the scores stream to the filler's own
        # exp deps through the in-order PE queue and starves ACT.
        nk = 4 * qi + 4
        pTs = []
        L = len(fillers)
        for s in range(nk):
            emit_scores_step(qi, ht, s, pTs)
            for f in fillers[L * s // nk: L * (s + 1) // nk]:
                f()
        return pTs

    def interleave(a, b):
        res = []
        n = max(len(a), len(b))
        for i in range(n):
            if i < len(a):
                res.append(a[i])
            if i < len(b):
                res.append(b[i])
        return res

    # ---------------- schedule ----------------
    # phase 1: q/k for ht=0 (DMA-paced)
    emit_qk_phase0()

    qk1 = qk1_chunk_items()  # deferred: run as loop filler in it1-it3
    h0ref, h1ref = {}, {}
    # iteration order (qi, ht); extras sized against each iteration's exp load
    order = [(0, 0), (1, 0), (2, 0), (3, 0), (3, 1), (2, 1), (1, 1), (0, 1)]
    prev = None
    for it, (qi, ht) in enumerate(order):
        fillers = []
        av_pre = []
        if prev is not None:
            av, zref = av_items(prev["qi"], prev["ht"], prev["pTs"])
            av_pre = av + norm_items(prev["qi"], prev["ht"], zref)
        if it == 0:
            fillers = v_group_items(0) + v_group_items(1)
        elif it == 1:
            fillers = av_pre + v_group_items(2) + v_group_items(3) + qk1[0:2]
        elif it == 2:
            fillers = av_pre + v_group_items(4) + v_group_items(5) + qk1[2:5]
        elif it == 3:
            fillers = av_pre + v_group_items(6) + qk1[5:8]
        elif it == 4:
            # v group 7 (k-tiles 14-15) must land before AV(3,0) reaches
            # k-tiles 14-15 (item index 14); interleaving keeps PE fed while
            # ACT chews the 16 exps.
            fillers = (interleave(av_pre[0:10], v_group_items(7))
                       + av_pre[10:] + proj_half0_items(0, h0ref))
        elif it == 5:
            fillers = av_pre + proj_items(3) + proj_half0_items(1, h1ref)
        elif it == 6:
            fillers = av_pre + proj_items(2)
        elif it == 7:
            fillers = av_pre + proj_half1_items(1, h1ref)
        pTs = emit_iteration(qi, ht, fillers)
        prev = {"qi": qi, "ht": ht, "pTs": pTs}

    # tail
    av, zref = av_items(prev["qi"], prev["ht"], prev["pTs"])
    for f in (av + norm_items(prev["qi"], prev["ht"], zref)
              + proj_half1_items(0, h0ref)):
        f()


def _build(bias_qkv):
    key = bool(bias_qkv)
    if key in _GRAPH_CACHE:
        return _GRAPH_CACHE[key]
    import contextlib

    nc = bacc.Bacc("TRN2", target_bir_lowering=False, debug=False, num_devices=8)
    with contextlib.ExitStack() as ctx:
        tc = ctx.enter_context(tile.TileContext(nc))
        _emit(nc, tc, ctx, bias_qkv)
    nc.compile()
    _GRAPH_CACHE[key] = nc
    return nc


def _make_masks():
    kl = np.arange(128)[:, None]
    ql = np.arange(128)[None, :]
    m = (kl <= ql).astype(np.float32)  # [128 k, 128 q] lower-tri incl diag
    m2 = np.stack([m, m], axis=1)      # duplicate for the two heads of a pair
    return np.ascontiguousarray(m2.reshape(128, 256)).astype(NPBF16)


def kernel(normalized_resid_pre, W_Q, W_K, W_V, W_O, b_Q, b_K, b_V, b_O):
    global LAST_RESULT
    x = np.asarray(normalized_resid_pre, dtype=np.float32)
    W_Q = np.asarray(W_Q, dtype=np.float32)
    W_K = np.asarray(W_K, dtype=np.float32)
    W_V = np.asarray(W_V, dtype=np.float32)
    W_O = np.asarray(W_O, dtype=np.float32)
    b_Q = np.asarray(b_Q, dtype=np.float32)
    b_K = np.asarray(b_K, dtype=np.float32)
    b_V = np.asarray(b_V, dtype=np.float32)
    b_O = np.asarray(b_O, dtype=np.float32)

    bias_qkv = bool(np.any(b_Q) or np.any(b_K) or np.any(b_V))
    nc = _build(bias_qkv)

    mask_np = _make_masks()
    xT = [np.ascontiguousarray(x[b].T).astype(NPBF16) for b in range(B)]

    in_maps = []
    for c in range(8):
        b, g = c // 4, c % 4
        hs = slice(4 * g, 4 * g + 4)
        m = {
            "xT": xT[b],
            "wq": np.ascontiguousarray(
                W_Q[hs].transpose(1, 0, 2).reshape(D, HE)).astype(NPBF16),
            "wk": np.ascontiguousarray(
                W_K[hs].transpose(1, 0, 2).reshape(D, HE)).astype(NPBF16),
            "wv": np.ascontiguousarray(
                W_V[hs].transpose(1, 0, 2).reshape(D, HE)).astype(NPBF16),
            "wo": np.ascontiguousarray(W_O[hs].reshape(HE, D)).astype(NPBF16),
            "masks": mask_np,
        }
        if bias_qkv:
            m["bq"] = np.ascontiguousarray(b_Q[hs].reshape(HE))
            m["bk"] = np.ascontiguousarray(b_K[hs].reshape(HE))
            m["bv"] = np.ascontiguousarray(b_V[hs].reshape(HE))
        in_maps.append(m)

    res = run_bass_kernel_spmd(nc, in_maps, list(range(8)))
    LAST_RESULT = res

    full = np.zeros((B, S, D), dtype=np.float32)
    for c in range(8):
        o = np.asarray(res.results[c]["out"])
        if o.dtype != np.float32:
            o = o.view(ml_dtypes.bfloat16) if o.dtype.kind == "V" else o
            o = o.astype(np.float32)
        full[c // 4] += o.T
    full += b_O[None, None, :]
    return full


# revision 22
# speedup vs baseline: 1.0236x; 1.0236x over previous
"""Causal multi-head attention on 8 TRN2 NeuronCores.

Sharding: core c -> (batch b = c // 4, head-group g = c % 4, heads 4g..4g+3).
Each core computes its 4 heads' attention plus the partial output projection
(out_partial^T = W_O_g^T @ z_g^T, [1024, 2048] fp32). Host sums the 4 head-group
partials per batch, adds b_O, transposes back.

v9 schedule (v8 was 168.7us):
  - Dual-ring input DMA: xT quarters on the Sync HWDGE ring, weights on the
    Activation HWDGE ring, with the first wk/wq/xT-q0 transfers split in
    halves so the ht=0 qi=0 q/k chains start ~5us earlier.
  - PE p-state warmup: ~8 dummy matmuls on zero tiles during the DMA window
    so the first real matmul runs at full 2.4GHz (idle PE drops to 1.2GHz and
    needs ~3us of continuous work to ramp back).
  - Causal mask folded into the scores PSUM accumulation as an additive
    -1e5 matmul (stationary=I, moving=Mneg, start=False) on each diagonal
    k-tile: exp feeds AV directly, GpSimd is out of the critical path.
  - Out-projection PSUM->SBUF casts on DVE instead of ACT (ACT was the
    co-bottleneck at 135us busy with exp=75us + identity=21us).
  - Denominator broadcast as ONE matmul: v_aug has ones-columns 64 (hb0) and
    65 (hb1), so den0/den1 land on PSUM rows 64/65 of the two zpair banks;
    two lane-aligned copies then a single [2,128]-stationary matmul
    broadcasts both into one [128, 512] PSUM tile; one reciprocal, 2 mults.
  - PSUM: sc [128,2,512]x2 | zpair [128,2,512]x1 | ms [128,512]x2 = 8 banks.
    Hard-won rule: matmul start=True clears the whole bank row of every
    partition it writes, so interleaved accumulation chains may share a bank
    only at disjoint PARTITION ranges, never disjoint column ranges.
  - Per step: scores FIRST (only waits on the spool slot, i.e. ACT pace),
    then fillers - a filler emitted before scores couples the scores stream
    to the filler's exp deps through the in-order PE queue.
"""

import os
import sys

import numpy as np

for _p in ("/opt/trn_rl_repo", "/opt/pypackages"):
    if _p not in sys.path and os.path.isdir(_p):
        sys.path.append(_p)

import ml_dtypes  # noqa: E402

import concourse.bass as bass  # noqa: E402
import concourse.mybir as mybir  # noqa: E402
import concourse.tile as tile  # noqa: E402
from concourse import bacc  # noqa: E402
from concourse.bass_utils import run_bass_kernel_spmd  # noqa: E402

BF16 = mybir.dt.bfloat16
F32 = mybir.dt.float32
F32R = mybir.dt.float32r
NPBF16 = ml_dtypes.bfloat16

B = 2
S = 2048
D = 1024
N_HEADS = 16
DH = 64
NH_CORE = 4          # heads per core
HE = NH_CORE * DH    # 256 concatenated head dims per core
QB = 512             # q block (moving operand width)
NQ = S // QB         # 4
NKT = S // 128       # 16 k-position tiles
NDT = D // 128       # 8 d_model tiles

LAST_RESULT = None
_GRAPH_CACHE = {}


def _emit(nc, tc, ctx, bias_qkv):
    xT = nc.declare_dram_parameter("xT", [D, S], BF16, isOutput=False).ap()
    wq = nc.declare_dram_parameter("wq", [D, HE], BF16, isOutput=False).ap()
    wk = nc.declare_dram_parameter("wk", [D, HE], BF16, isOutput=False).ap()
    wv = nc.declare_dram_parameter("wv", [D, HE], BF16, isOutput=False).ap()
    wo = nc.declare_dram_parameter("wo", [HE, D], BF16, isOutput=False).ap()
    masks = nc.declare_dram_parameter("masks", [128, 2 * 128], BF16, isOutput=False).ap()
    if bias_qkv:
        bq = nc.declare_dram_parameter("bq", [HE], F32, isOutput=False).ap()
        bk = nc.declare_dram_parameter("bk", [HE], F32, isOutput=False).ap()
        bv = nc.declare_dram_parameter("bv", [HE], F32, isOutput=False).ap()
    out = nc.declare_dram_parameter("out", [D, S], BF16, isOutput=True).ap()

    consts = ctx.enter_context(tc.tile_pool(name="consts", bufs=1))

    xT_sb = consts.tile([128, NDT, S], BF16)
    wq_sb = consts.tile([128, NDT, HE], BF16)
    wk_sb = consts.tile([128, NDT, HE], BF16)
    wv_sb = consts.tile([128, NDT, HE], BF16)
    wo_sb = consts.tile([128, 2, D], BF16)
    mask2_sb = consts.tile([128, 2, 128], BF16)

    # Activation-ring DMAs: consts + weights, first-needed split in halves.
    nc.scalar.dma_start(out=mask2_sb, in_=masks)
    nc.scalar.dma_start(out=wk_sb[:, 0:4, :],
                        in_=wk[0:512, :].rearrange("(t p) e -> p t e", p=128))
    nc.scalar.dma_start(out=wk_sb[:, 4:8, :],
                        in_=wk[512:1024, :].rearrange("(t p) e -> p t e", p=128))
    nc.scalar.dma_start(out=wq_sb[:, 0:4, :],
                        in_=wq[0:512, :].rearrange("(t p) e -> p t e", p=128))
    nc.scalar.dma_start(out=wq_sb[:, 4:8, :],
                        in_=wq[512:1024, :].rearrange("(t p) e -> p t e", p=128))

    # Sync-ring DMAs: xT in sequence-column quarters (q0 in d-halves).
    nc.sync.dma_start(out=xT_sb[:, 0:4, 0:QB],
                      in_=xT[0:512, 0:QB].rearrange("(t p) s -> p t s", p=128))
    nc.sync.dma_start(out=xT_sb[:, 4:8, 0:QB],
                      in_=xT[512:1024, 0:QB].rearrange("(t p) s -> p t s", p=128))
    for qs in range(1, 4):
        nc.sync.dma_start(
            out=xT_sb[:, :, qs * QB:(qs + 1) * QB],
            in_=xT[:, qs * QB:(qs + 1) * QB].rearrange("(t p) s -> p t s", p=128))

    # ACT exp table pre-load (~2.7us) during the input-DMA window, after the
    # first weight-DMA issues so it doesn't delay them on the ACT queue.
    warm_i = consts.tile([128, 8], F32)
    nc.vector.memset(warm_i, 0.0)
    warm_o = consts.tile([128, 8], BF16)
    nc.scalar.activation(out=warm_o, in_=warm_i,
                         func=mybir.ActivationFunctionType.Exp, scale=0.125)

    nc.scalar.dma_start(out=wv_sb, in_=wv.rearrange("(t p) e -> p t e", p=128))
    nc.scalar.dma_start(out=wo_sb, in_=wo.rearrange("(t p) d -> p t d", p=128))

    if bias_qkv:
        bq_sb = consts.tile([128, 2], F32)
        nc.scalar.dma_start(out=bq_sb, in_=bq.rearrange("(t p) -> p t", p=128))
        bk_sb = consts.tile([128, 2], F32)
        nc.scalar.dma_start(out=bk_sb, in_=bk.rearrange("(t p) -> p t", p=128))
        bv_r = bv.rearrange("(a b e) -> a b e", a=2, b=2)
        bv_bcast = bass.AP(tensor=bv_r.tensor, offset=bv_r.offset,
                           ap=[[0, 128]] + list(bv_r.ap))
        bv_sb = consts.tile([128, 2, 2, DH], F32)
        nc.scalar.dma_start(out=bv_sb, in_=bv_bcast)

    qT_sb = consts.tile([128, 2, S], BF16)
    kT_sb = consts.tile([128, 2, S], BF16)
    zT_sb = consts.tile([128, 2, S], BF16)
    # v_aug [k, ht, hb, 65]: cols 0-63 = v, col 64 = 1 -> each head's softmax
    # denominator accumulates on PSUM row 64 of its own zpair bank.
    VA = DH + 1
    v_sb = consts.tile([128, NKT, 2, 2, VA], BF16)
    nc.vector.memset(v_sb[:, :, :, :, DH:VA], 1.0)

    # ones row (partition 64) for the denominator broadcast matmuls
    ones64 = consts.tile([128, DH], BF16)
    nc.vector.memset(ones64[64:65, :], 1.0)

    # PE p-state warmup fodder (zeros; overwritten semantics-free).
    warm_w = consts.tile([128, 128], BF16)
    nc.vector.memset(warm_w, 0.0)
    warm_m = consts.tile([128, QB], BF16)
    nc.vector.memset(warm_m, 0.0)

    # PSUM: sc 2x[128,2,512] (4 banks) + zpair 1x[128,2,512] (2) +
    # misc 2x[128,512] (2) = 8 banks.
    spool = ctx.enter_context(tc.tile_pool(name="spool", bufs=2, space="PSUM"))
    zpool = ctx.enter_context(tc.tile_pool(name="zpool", bufs=1, space="PSUM"))
    mpool = ctx.enter_context(tc.tile_pool(name="mpool", bufs=2, space="PSUM"))
    ppool = ctx.enter_context(tc.tile_pool(name="ppool", bufs=32))
    rpool = ctx.enter_context(tc.tile_pool(name="rpool", bufs=2))
    opool = ctx.enter_context(tc.tile_pool(name="opool", bufs=6))
    hpool = ctx.enter_context(tc.tile_pool(name="hpool", bufs=16))

    # ---------------- q/k projections ----------------
    def emit_qk_phase0():
        """ht=0 q/k chains emitted per sequence quarter, in the quarter's xT
        arrival order; each chain's cast follows immediately so the first
        scores/exp of the attention loop start while xT is still landing.
        Chain homes: qi0/qi1 -> sc pool, qi2 -> zpair, qi3 -> 2 ms tiles, so
        the scores(0,0) spool request only waits on qi0's cast."""
        hsl = slice(0, 128)

        def cast(dst, src_ap, bias_sb, qi):
            if bias_qkv:
                nc.scalar.activation(
                    out=dst[:, 0, qi * QB:(qi + 1) * QB], in_=src_ap,
                    func=mybir.ActivationFunctionType.Identity,
                    bias=bias_sb[:, 0:1])
            else:
                nc.vector.tensor_copy(
                    out=dst[:, 0, qi * QB:(qi + 1) * QB], in_=src_ap)

        for qi in range(NQ):
            if qi < 2:
                ch = spool.tile([128, 2, QB], F32, tag="sc", name=f"qk0c{qi}")
                kch, qch = ch[:, 0, :], ch[:, 1, :]
            elif qi == 2:
                ch = zpool.tile([128, 2, QB], F32, tag="zp", name="qk0c2")
                kch, qch = ch[:, 0, :], ch[:, 1, :]
            else:
                kch = mpool.tile([128, QB], F32, tag="ms", name="qk0c3k")
                qch = mpool.tile([128, QB], F32, tag="ms", name="qk0c3q")
            if qi == 0:
                # p-state warmup A: dummy matmuls with no DMA deps fill the
                # queue until the first wk/xT halves land.
                for wi in range(11):
                    nc.tensor.matmul(kch, warm_w, warm_m,
                                     start=(wi == 0), stop=(wi == 10))
            qsl = slice(qi * QB, (qi + 1) * QB)
            for t in range(4):
                nc.tensor.matmul(kch, wk_sb[:, t, hsl], xT_sb[:, t, qsl],
                                 start=(t == 0), stop=False)
            if qi == 0:
                # warmup B in the qch bank: fills the DMA wait between the
                # first xT/wk half and the second.
                for wi in range(5):
                    nc.tensor.matmul(qch, warm_w, warm_m,
                                     start=(wi == 0), stop=(wi == 4))
            for t in range(4, NDT):
                nc.tensor.matmul(kch, wk_sb[:, t, hsl], xT_sb[:, t, qsl],
                                 start=False, stop=(t == NDT - 1))
            cast(kT_sb, kch, bk_sb if bias_qkv else None, qi)
            for t in range(NDT):
                nc.tensor.matmul(qch, wq_sb[:, t, hsl], xT_sb[:, t, qsl],
                                 start=(t == 0), stop=(t == NDT - 1))
            cast(qT_sb, qch, bq_sb if bias_qkv else None, qi)

    def qk1_chunk_items():
        """ht=1 q/k chains as 8 short-lived contiguous chunks (one ms-pool
        tile each: 8 accumulating matmuls + an immediate cast). Short tile
        lifetime keeps the ms-pool cycling deadlock-free while these run as
        PE filler inside the attention loop (xT is fully resident by then)."""
        hsl = slice(128, 256)
        items = []
        for typ in ("k", "q"):
            for qi in range(NQ):
                def mk(typ=typ, qi=qi):
                    def f():
                        wsb = wk_sb if typ == "k" else wq_sb
                        dst = kT_sb if typ == "k" else qT_sb
                        ch = mpool.tile([128, QB], F32, tag="ms",
                                        name=f"qk1{typ}{qi}")
                        for t in range(NDT):
                            nc.tensor.matmul(
                                ch, wsb[:, t, hsl],
                                xT_sb[:, t, qi * QB:(qi + 1) * QB],
                                start=(t == 0), stop=(t == NDT - 1))
                        if bias_qkv:
                            bsb = bk_sb if typ == "k" else bq_sb
                            nc.scalar.activation(
                                out=dst[:, 1, qi * QB:(qi + 1) * QB], in_=ch,
                                func=mybir.ActivationFunctionType.Identity,
                                bias=bsb[:, 1:2])
                        else:
                            nc.vector.tensor_copy(
                                out=dst[:, 1, qi * QB:(qi + 1) * QB], in_=ch)
                    return f
                items.append(mk())
        return items

    # ---------------- V projection, per 2-k-tile group, t-outer ----------------
    # One PSUM bank per chain: matmul start=True clears the ENTIRE bank row of
    # every partition it writes, so two interleaved accumulation chains must
    # never share a bank in disjoint column ranges (disjoint partitions is ok).
    def v_group_items(g):
        tiles = {}
        items = []
        for t in range(NDT):
            def mk(t=t):
                def f():
                    if t == 0:
                        tiles[0] = mpool.tile([128, QB], F32, tag="ms",
                                              name=f"vg{g}a")
                        tiles[1] = mpool.tile([128, QB], F32, tag="ms",
                                              name=f"vg{g}b")
                    for i in range(2):
                        kt = 2 * g + i
                        nc.tensor.matmul(
                            tiles[i][:, 0:HE],
                            xT_sb[:, t, kt * 128:(kt + 1) * 128],
                            wv_sb[:, t, :],
                            start=(t == 0), stop=(t == NDT - 1))
                return f
            items.append(mk())

        def fcopy():
            for i in range(2):
                kt = 2 * g + i
                src = tiles[i][:, 0:HE].rearrange("p (a b e) -> p a b e",
                                                  a=2, b=2)
                if bias_qkv:
                    nc.vector.tensor_add(v_sb[:, kt, :, :, 0:DH], src, bv_sb)
                else:
                    nc.vector.tensor_copy(out=v_sb[:, kt, :, :, 0:DH], in_=src)
        items.append(fcopy)
        return items

    # ---------------- attention pieces ----------------
    def emit_scores_step(qi, ht, kj, pTs):
        qo = max(0, kj - 4 * qi) * 128
        ps = spool.tile([128, 2, QB], F32, tag="sc", name=f"sc{qi}{ht}{kj}")
        for hb in range(2):
            pb = hb * 64
            nc.tensor.matmul(
                ps[:, hb, qo:],
                kT_sb[pb:pb + 64, ht, kj * 128:(kj + 1) * 128],
                qT_sb[pb:pb + 64, ht, qi * QB + qo:(qi + 1) * QB],
                start=True, stop=True)
        pT = ppool.tile([128, 2, QB], BF16, tag="pT", name=f"pT{qi}{ht}{kj}")
        nc.scalar.activation(
            out=pT[:, :, qo:], in_=ps[:, :, qo:],
            func=mybir.ActivationFunctionType.Exp, scale=0.125)
        if kj >= 4 * qi:
            # causal mask only bites in the [128, 128] square at the diagonal
            # (GpSimd has slack; AV consumes pT a full iteration later, so
            # the extra latency hop is hidden)
            nc.gpsimd.tensor_mul(
                pT[:, :, qo:qo + 128], pT[:, :, qo:qo + 128], mask2_sb)
        pTs.append((pT, qo))

    def av_items(qi, ht, pTs):
        nk = 4 * qi + 4
        zref = {}
        items = []
        for kj in range(nk):
            def mk(kj=kj):
                def f():
                    if kj == 0:
                        zref["z"] = zpool.tile([128, 2, QB], F32, tag="zp",
                                               name=f"zp{qi}{ht}")
                    zp = zref["z"]
                    pT, qo = pTs[kj]
                    for hb in range(2):
                        nc.tensor.matmul(
                            zp[0:VA, hb, qo:],
                            v_sb[:, kj, ht, hb, 0:VA],
                            pT[:, hb, qo:],
                            start=(kj == 0), stop=(kj == nk - 1))
                return f
            items.append(mk())
        return items, zref

    def norm_items(qi, ht, zref):
        qsl = slice(qi * QB, (qi + 1) * QB)
        rref = {}

        # bf16 copy of z+den rows per head-pair half: the copies become zp's
        # ONLY readers, so the zpair PSUM banks free ~1.4us earlier and
        # AV(i+1) (zpool bufs=1) starts while the norm math proceeds on the
        # SBUF copy. Split per-hb for finer PE/DVE interleave. Everything is
        # lane-aligned at base 0 (SBUF x SBUF DVE ops require equal bases).
        def f_c0():
            zc = rpool.tile([128, 2, QB], BF16, tag="ds", name=f"zc{qi}{ht}")
            rref["d"] = zc
            nc.vector.tensor_copy(out=zc[0:DH + 1, 0, :],
                                  in_=zref["z"][0:DH + 1, 0, :])

        def f_ba():
            dpa = mpool.tile([128, QB], F32, tag="ms", name=f"dpa{qi}{ht}")
            rref["ba"] = dpa
            nc.tensor.matmul(dpa[0:DH, :], ones64[64:65, :],
                             rref["d"][64:65, 0, :], start=True, stop=True)

        def f_c1():
            nc.vector.tensor_copy(out=rref["d"][0:DH + 1, 1, :],
                                  in_=zref["z"][0:DH + 1, 1, :])

        def f_bb():
            dpb = mpool.tile([128, QB], F32, tag="ms", name=f"dpb{qi}{ht}")
            rref["bb"] = dpb
            nc.tensor.matmul(dpb[0:DH, :], ones64[64:65, :],
                             rref["d"][64:65, 1, :], start=True, stop=True)

        def f_m0():
            zc = rref["d"]
            bsb = rpool.tile([128, 2, QB], F32, tag="bs", name=f"bsb{qi}{ht}")
            rref["r"] = bsb
            nc.vector.reciprocal_approx_fast(bsb[0:DH, 0, :],
                                             rref["ba"][0:DH, :])
            nc.vector.tensor_mul(zT_sb[0:DH, ht, qsl], zc[0:DH, 0, :],
                                 bsb[0:DH, 0, :])

        def f_m1():
            zc = rref["d"]
            bsb = rref["r"]
            nc.vector.reciprocal_approx_fast(bsb[0:DH, 1, :],
                                             rref["bb"][0:DH, :])
            nc.vector.tensor_mul(zT_sb[DH:2 * DH, ht, qsl], zc[0:DH, 1, :],
                                 bsb[0:DH, 1, :])
        return [f_c0, f_ba, f_c1, f_bb, f_m0, f_m1]

    def proj_items(qi):
        qsl = slice(qi * QB, (qi + 1) * QB)
        items = []
        for dt in range(NDT):
            def mk(dt=dt):
                def f():
                    ops = mpool.tile([128, QB], F32, tag="ms",
                                     name=f"pj{qi}{dt}")
                    for t in range(2):
                        nc.tensor.matmul(
                            ops, wo_sb[:, t, dt * 128:(dt + 1) * 128],
                            zT_sb[:, t, qsl],
                            start=(t == 0), stop=(t == 1))
                    osb = opool.tile([128, QB], BF16, tag="ot",
                                     name=f"ot{qi}{dt}")
                    # alternate casts between DVE and ACT so neither queue
                    # backs up ahead of the norm chain
                    if dt % 2 == 0:
                        nc.vector.tensor_copy(out=osb, in_=ops)
                    else:
                        nc.scalar.activation(
                            out=osb, in_=ops,
                            func=mybir.ActivationFunctionType.Identity)
                    nc.sync.dma_start(out=out[dt * 128:(dt + 1) * 128, qsl],
                                      in_=osb)
                return f
            items.append(mk())
        return items

    # ht-split projection for the work-starved last iterations: the ht=0
    # half depends only on norm(qi,0) (done phases earlier), so it runs as
    # mid-kernel filler into SBUF bf16; the late half is then a single
    # matmul + fused add-cast per output tile.
    def proj_half0_items(qi, href):
        qsl = slice(qi * QB, (qi + 1) * QB)
        items = []
        for dt in range(NDT):
            def mk(dt=dt):
                def f():
                    hps = mpool.tile([128, QB], F32, tag="ms",
                                     name=f"ph{qi}{dt}")
                    nc.tensor.matmul(
                        hps, wo_sb[:, 0, dt * 128:(dt + 1) * 128],
                        zT_sb[:, 0, qsl], start=True, stop=True)
                    hsb = hpool.tile([128, QB], BF16, tag="hf",
                                     name=f"hf{qi}{dt}")
                    href[dt] = hsb
                    if dt % 2 == 0:
                        nc.vector.tensor_copy(out=hsb, in_=hps)
                    else:
                        nc.scalar.activation(
                            out=hsb, in_=hps,
                            func=mybir.ActivationFunctionType.Identity)
                return f
            items.append(mk())
        return items

    def proj_half1_items(qi, href):
        qsl = slice(qi * QB, (qi + 1) * QB)
        items = []
        for dt in range(NDT):
            def mk(dt=dt):
                def f():
                    ops = mpool.tile([128, QB], F32, tag="ms",
                                     name=f"pk{qi}{dt}")
                    nc.tensor.matmul(
                        ops, wo_sb[:, 1, dt * 128:(dt + 1) * 128],
                        zT_sb[:, 1, qsl], start=True, stop=True)
                    osb = opool.tile([128, QB], BF16, tag="ot",
                                     name=f"ot{qi}{dt}")
                    nc.vector.tensor_add(osb, ops, href[dt])
                    nc.sync.dma_start(out=out[dt * 128:(dt + 1) * 128, qsl],
                                      in_=osb)
                return f
            items.append(mk())
        return items

    def emit_iteration(qi, ht, fillers):
        # scores FIRST in each step: scores(s) only waits on the spool slot
        # (exp(s-2), i.e. ACT pace), keeping ACT fed; the fillers behind it
        # (AV(i-1)/projections/V) are dependency-clean by then. Emitting a
        # filler before scores couples the scores stream to the filler's own
        # exp deps through the in-order PE queue and starves ACT.
        nk = 4 * qi + 4
        pTs = []
        L = len(fillers)
        for s in range(nk):
            emit_scores_step(qi, ht, s, pTs)
            for f in fillers[L * s // nk: L * (s + 1) // nk]:
                f()
        return pTs

    def interleave(a, b):
        res = []
        n = max(len(a), len(b))
        for i in range(n):
            if i < len(a):
                res.append(a[i])
            if i < len(b):
                res.append(b[i])
        return res

    # ---------------- schedule ----------------
    # phase 1: q/k for ht=0 (DMA-paced)
    emit_qk_phase0()

    qk1 = qk1_chunk_items()  # deferred: run as loop filler in it1-it3
    h0ref, h1ref = {}, {}
    # iteration order (qi, ht); extras sized against each iteration's exp load
    order = [(0, 0), (1, 0), (2, 0), (3, 0), (3, 1), (2, 1), (1, 1), (0, 1)]
    prev = None
    for it, (qi, ht) in enumerate(order):
        fillers = []
        av_pre = []
        if prev is not None:
            av, zref = av_items(prev["qi"], prev["ht"], prev["pTs"])
            av_pre = av + norm_items(prev["qi"], prev["ht"], zref)
        if it == 0:
            fillers = v_group_items(0) + v_group_items(1)
        elif it == 1:
            fillers = av_pre + v_group_items(2) + v_group_items(3) + qk1[0:2]
        elif it == 2:
            fillers = av_pre + v_group_items(4) + v_group_items(5) + qk1[2:5]
        elif it == 3:
            fillers = av_pre + v_group_items(6) + qk1[5:8]
        elif it == 4:
            # v group 7 (k-tiles 14-15) must land before AV(3,0) reaches
            # k-tiles 14-15 (item index 14); interleaving keeps PE fed while
            # ACT chews the 16 exps.
            fillers = interleave(av_pre[0:10], v_group_items(7)) + av_pre[10:]
        elif it == 5:
            fillers = av_pre + proj_items(3)
        elif it == 6:
            fillers = av_pre + proj_items(2)
        elif it == 7:
            fillers = av_pre + proj_items(1)
        pTs = emit_iteration(qi, ht, fillers)
        prev = {"qi": qi, "ht": ht, "pTs": pTs}

    # tail
    av, zref = av_items(prev["qi"], prev["ht"], prev["pTs"])
    for f in av + norm_items(prev["qi"], prev["ht"], zref) + proj_items(0):
        f()


def _build(bias_qkv):
    key = bool(bias_qkv)
    if key in _GRAPH_CACHE:
        return _GRAPH_CACHE[key]
    import contextlib

    nc = bacc.Bacc("TRN2", target_bir_lowering=False, debug=False, num_devices=8)
    with contextlib.ExitStack() as ctx:
        tc = ctx.enter_context(tile.TileContext(nc))
        _emit(nc, tc, ctx, bias_qkv)
    nc.compile()
    _GRAPH_CACHE[key] = nc
    return nc


def _make_masks():
    kl = np.arange(128)[:, None]
    ql = np.arange(128)[None, :]
    m = (kl <= ql).astype(np.float32)  # [128 k, 128 q] lower-tri incl diag
    m2 = np.stack([m, m], axis=1)      # duplicate for the two heads of a pair
    return np.ascontiguousarray(m2.reshape(128, 256)).astype(NPBF16)


def kernel(normalized_resid_pre, W_Q, W_K, W_V, W_O, b_Q, b_K, b_V, b_O):
    global LAST_RESULT
    x = np.asarray(normalized_resid_pre, dtype=np.float32)
    W_Q = np.asarray(W_Q, dtype=np.float32)
    W_K = np.asarray(W_K, dtype=np.float32)
    W_V = np.asarray(W_V, dtype=np.float32)
    W_O = np.asarray(W_O, dtype=np.float32)
    b_Q = np.asarray(b_Q, dtype=np.float32)
    b_K = np.asarray(b_K, dtype=np.float32)
    b_V = np.asarray(b_V, dtype=np.float32)
    b_O = np.asarray(b_O, dtype=np.float32)

    bias_qkv = bool(np.any(b_Q) or np.any(b_K) or np.any(b_V))
    nc = _build(bias_qkv)

    mask_np = _make_masks()
    xT = [np.ascontiguousarray(x[b].T).astype(NPBF16) for b in range(B)]

    in_maps = []
    for c in range(8):
        b, g = c // 4, c % 4
        hs = slice(4 * g, 4 * g + 4)
        m = {
            "xT": xT[b],
            "wq": np.ascontiguousarray(
                W_Q[hs].transpose(1, 0, 2).reshape(D, HE)).astype(NPBF16),
            "wk": np.ascontiguousarray(
                W_K[hs].transpose(1, 0, 2).reshape(D, HE)).astype(NPBF16),
            "wv": np.ascontiguousarray(
                W_V[hs].transpose(1, 0, 2).reshape(D, HE)).astype(NPBF16),
            "wo": np.ascontiguousarray(W_O[hs].reshape(HE, D)).astype(NPBF16),
            "masks": mask_np,
        }
        if bias_qkv:
            m["bq"] = np.ascontiguousarray(b_Q[hs].reshape(HE))
            m["bk"] = np.ascontiguousarray(b_K[hs].reshape(HE))
            m["bv"] = np.ascontiguousarray(b_V[hs].reshape(HE))
        in_maps.append(m)

    res = run_bass_kernel_spmd(nc, in_maps, list(range(8)))
    LAST_RESULT = res

    full = np.zeros((B, S, D), dtype=np.float32)
    for c in range(8):
        o = np.asarray(res.results[c]["out"])
        if o.dtype != np.float32:
            o = o.view(ml_dtypes.bfloat16) if o.dtype.kind == "V" else o
            o = o.astype(np.float32)
        full[c // 4] += o.T
    full += b_O[None, None, :]
    return full
